# revision 1
# baseline (speedup 1.0000x reference)
"""Multi-head causal self-attention (B=128, T=256, C=384, H=6, HS=64) for 8 TRN2 cores.

Strategy: pure data-parallel over batch (16 batch elements per core), weights
replicated, no collectives. Per batch element:

  - x^T (pre-transposed on host, [C, T]) is the shared rhs/lhsT for projections
  - Q^T, K^T computed per head-pair as [128(d), 256(t)] PSUM tiles (N=256 matmuls)
  - V computed in natural [t, (h d)] layout (rhs = all heads at once, N=384)
  - scores = Q^T.T-slices @ K^T with causal block-skipping:
      block(0,0) triangular [128,128], block(1,0) full, block(1,1) triangular;
      block(0,1) is never computed.
  - softmax without max-subtraction (scores bounded for this distribution):
      exp on ACT (one op per head over the packed [128, 384] score tile),
      causal mask applied multiplicatively fused with the row-sum
      (tensor_tensor_reduce), then normalize with per-partition reciprocal.
  - P transposed via PE (3x [128,128] per head) for the AV matmul,
    AV accumulated as [d, t] directly into the concat-head layout att^T
  - y = att^T.T @ Wp^T + bp, bias fused into the PSUM->SBUF copy on DVE.

Matmul operands in bf16 (fp32 PSUM accumulation), softmax stats in fp32.
"""

import numpy as np
import ml_dtypes
from contextlib import ExitStack

import concourse.bass as bass
import concourse.bacc as bacc
import concourse.mybir as mybir
import concourse.tile as tile
from concourse.bass_utils import run_bass_kernel_spmd

B, T, C, H, HS = 128, 256, 384, 6, 64
NCORES = 8
BPC = B // NCORES  # batch elements per core

F32 = mybir.dt.float32
DT = mybir.dt.bfloat16
NPDT = ml_dtypes.bfloat16

EXP = mybir.ActivationFunctionType.Exp
MUL = mybir.AluOpType.mult
ADD = mybir.AluOpType.add


def build(n_batch: int = BPC) -> bass.Bass:
    assert n_batch % 2 == 0
    npair = n_batch // 2
    nc = bacc.Bacc("TRN2", target_bir_lowering=False, debug=False)

    xT = nc.dram_tensor("xT", [npair, 3, 128, 2 * T], DT, kind="ExternalInput").ap()
    wq = nc.dram_tensor("wq", [128, 3, 3, 128], DT, kind="ExternalInput").ap()
    wk = nc.dram_tensor("wk", [128, 3, 3, 128], DT, kind="ExternalInput").ap()
    wv = nc.dram_tensor("wv", [128, 3, C], DT, kind="ExternalInput").ap()
    wp = nc.dram_tensor("wp", [128, 3, C], DT, kind="ExternalInput").ap()
    msk = nc.dram_tensor("msk", [128, 128], DT, kind="ExternalInput").ap()
    bb = nc.dram_tensor("bb", [128, C], F32, kind="ExternalInput").ap()
    y = nc.dram_tensor("y", [n_batch, T, C], F32, kind="ExternalOutput").ap()

    with tile.TileContext(nc) as tc, ExitStack() as ctx:
        const = ctx.enter_context(tc.tile_pool(name="const", bufs=1))
        sb = ctx.enter_context(tc.tile_pool(name="sb", bufs=2))
        # uniform pool: every PSUM tile here is <= 1 bank
        psa = ctx.enter_context(tc.tile_pool(name="psa", bufs=8, space="PSUM"))

        wq_t = const.tile([128, 3, 3, 128], DT)
        nc.sync.dma_start(out=wq_t, in_=wq)
        wk_t = const.tile([128, 3, 3, 128], DT)
        nc.sync.dma_start(out=wk_t, in_=wk)
        wv_t = const.tile([128, 3, C], DT)
        nc.gpsimd.dma_start(out=wv_t, in_=wv)
        wp_t = const.tile([128, 3, C], DT)
        nc.gpsimd.dma_start(out=wp_t, in_=wp)
        msk_t = const.tile([128, 128], DT)
        nc.gpsimd.dma_start(out=msk_t, in_=msk)
        bb_t = const.tile([128, C], F32)
        nc.gpsimd.dma_start(out=bb_t, in_=bb)
        ones_col = const.tile([128, 1], DT)
        nc.vector.memset(ones_col, 1.0)
        ones_row = const.tile([1, 128], DT)
        nc.vector.memset(ones_row, 1.0)

        # HAM warm-up: ~7us of back-to-back dummy matmuls during the initial
        # weight/x DMA wait, so the PE clock is at 8/8 when real work starts.
        # Same-engine WAW chain -> no cross-engine waits, PE runs them densely.
        warm_in = const.tile([128, 512], DT)
        nc.vector.memset(warm_in, 0.0)
        warm_ps = psa.tile([128, 512], F32, tag="att")
        for _ in range(30):
            nc.tensor.matmul(
                warm_ps, lhsT=warm_in[:, 0:128], rhs=warm_in,
                start=True, stop=True,
            )

        prev = None
        for pair in range(npair):
            xt = sb.tile([128, 3, 2 * T], DT, tag="xt", bufs=4)
            nc.sync.dma_start(out=xt, in_=xT[pair].rearrange("k c t -> c k t"))

            # Q^T / K^T for both batch elems of the pair (N=512), per head
            # pair; V per batch elem in natural [t, (h d)] layout
            qt = sb.tile([128, 3, 2 * T], DT, tag="qt")
            kt = sb.tile([128, 3, 2 * T], DT, tag="kt")
            for p in range(3):
                qt_ps = psa.tile([128, 2 * T], F32, tag="att")
                for k in range(3):
                    nc.tensor.matmul(
                        qt_ps,
                        lhsT=wq_t[:, k, p, :],
                        rhs=xt[:, k, :],
                        start=(k == 0),
                        stop=(k == 2),
                    )
                nc.scalar.copy(out=qt[:, p, :], in_=qt_ps)
                kt_ps = psa.tile([128, 2 * T], F32, tag="att")
                for k in range(3):
                    nc.tensor.matmul(
                        kt_ps,
                        lhsT=wk_t[:, k, p, :],
                        rhs=xt[:, k, :],
                        start=(k == 0),
                        stop=(k == 2),
                    )
                nc.scalar.copy(out=kt[:, p, :], in_=kt_ps)
            vs = []
            for bi in range(2):
                v = sb.tile([128, 2, C], DT, tag="v", bufs=6)
                for m in range(2):
                    v_ps = psa.tile([128, C], F32, tag="att")
                    for k in range(3):
                        nc.tensor.matmul(
                            v_ps,
                            lhsT=xt[:, k, bi * T + m * 128 : bi * T + (m + 1) * 128],
                            rhs=wv_t[:, k, :],
                            start=(k == 0),
                            stop=(k == 2),
                        )
                    nc.scalar.copy(out=v[:, m, :], in_=v_ps)
                vs.append(v)

            # stage A1: transposed scores st[s, tq] + causal mask + exp;
            # the two heads of a pair share one packed pex SBUF tile
            pexs = {}
            for bi in range(2):
                for pr in range(3):
                    pex = sb.tile([128, 2, 384], DT, tag="pex", bufs=10)
                    for two in range(2):
                        h = 2 * pr + two
                        lo = two * 64
                        qh = qt[lo : lo + 64, pr, bi * T : (bi + 1) * T]
                        kh = kt[lo : lo + 64, pr, bi * T : (bi + 1) * T]
                        # packed [s0 x tq(0:256) | s1 x tq(128:256)]
                        st = psa.tile([128, 384], F32, tag="att")
                        nc.tensor.matmul(
                            st[:, 0:256],
                            lhsT=kh[:, 0:128],
                            rhs=qh,
                            start=True,
                            stop=True,
                        )
                        nc.tensor.matmul(
                            st[:, 256:384],
                            lhsT=kh[:, 128:256],
                            rhs=qh[:, 128:256],
                            start=True,
                            stop=True,
                        )
                        # exp (scores bounded, no max trick); causal mask
                        # applied multiplicatively on pex below
                        nc.scalar.activation(out=pex[:, two, :], in_=st, func=EXP)
                    # multiplicative causal mask (keep tq >= s) on the four
                    # triangular blocks of the packed 2-head pex in ONE op:
                    # dims [p][head][block in {0:128, 256:384}][j]
                    pexv = bass.AP(
                        tensor=pex.tensor,
                        offset=pex.offset,
                        ap=[pex.ap[0], pex.ap[1], [256, 2], [1, 128]],
                    )
                    mskb = bass.AP(
                        tensor=msk_t.tensor,
                        offset=msk_t.offset,
                        ap=[msk_t.ap[0], [0, 2], [0, 2], msk_t.ap[1]],
                    )
                    nc.gpsimd.tensor_mul(out=pexv, in0=pexv, in1=mskb)
                    pexs[(bi, 2 * pr)] = pex[:, 0, :]
                    pexs[(bi, 2 * pr + 1)] = pex[:, 1, :]
                    pexs[(bi, "t", pr)] = pex

            # stage A2a: row sums (two matmuls per head pair) + approx
            # reciprocal + bf16 cast for all chains, so the later broadcast
            # matmuls never block the PE stream
            rrows = {}
            for bi in range(2):
                for pr in range(3):
                    pex2 = pexs[(bi, "t", pr)]  # [128, 2, 384]
                    sums = psa.tile([1, 512], F32, tag="att")
                    nc.tensor.matmul(
                        sums,
                        lhsT=ones_col,
                        rhs=pex2[:, :, 0:256],
                        start=True,
                        stop=False,
                        skip_group_check=True,
                    )
                    for two in range(2):
                        nc.tensor.matmul(
                            sums[:, two * 256 + 128 : two * 256 + 256],
                            lhsT=ones_col,
                            rhs=pex2[:, two, 256:384],
                            start=False,
                            stop=(two == 1),
                            skip_group_check=True,
                        )
                    rscr = sb.tile([1, 512], F32, tag="rscr", bufs=8)
                    nc.vector.reciprocal_approx_fast(out=rscr, in_=sums)
                    rrow = sb.tile([1, 512], DT, tag="rrow", bufs=8)
                    nc.vector.tensor_copy(out=rrow, in_=rscr)
                    rrows[(bi, pr)] = rrow

            # stage A2b: broadcast matmul + normalize for all chains
            pnorms = {}
            for bi in range(2):
                for pr in range(3):
                    pex2 = pexs[(bi, "t", pr)]
                    rrow = rrows[(bi, pr)]
                    bc = psa.tile([128, 512], F32, tag="att")
                    nc.tensor.matmul(
                        bc,
                        lhsT=ones_row,
                        rhs=rrow,
                        start=True,
                        stop=True,
                    )
                    bc_v = bc.rearrange("p (h j) -> p h j", j=256)
                    pnorm = sb.tile([128, 2, 384], DT, tag="pnorm", bufs=14)
                    nc.vector.tensor_mul(
                        out=pnorm[:, :, 0:256], in0=pex2[:, :, 0:256], in1=bc_v
                    )
                    nc.vector.tensor_mul(
                        out=pnorm[:, :, 256:384],
                        in0=pex2[:, :, 256:384],
                        in1=bc_v[:, :, 128:256],
                    )
                    pnorms[(bi, 2 * pr)] = pnorm[:, 0, :]
                    pnorms[(bi, 2 * pr + 1)] = pnorm[:, 1, :]


            def b_thunks(pair, vs, pnorms):
                # stage B split into 8 thunks: 6 head-pair AV chunks (+attT
                # copies) and 2 output projections, to interleave with the
                # next pair's A2b chains for a steady PE stream
                thunks = []
                attTs = {}

                def mk_av(bi, pr):
                    def th():
                        v = vs[bi]
                        if pr == 0:
                            attTs[bi] = sb.tile(
                                [128, 3, 256], DT, tag="attT", bufs=4, name=f"attT_{pair}_{bi}"
                            )
                        attT = attTs[bi]
                        av_ps = psa.tile([128, 256], F32, tag="att", name=f"av_{pair}_{bi}_{pr}")
                        for two in range(2):
                            h = 2 * pr + two
                            lo = two * 64
                            pnorm = pnorms[(bi, h)]
                            hs = slice(h * 64, h * 64 + 64)
                            nc.tensor.matmul(
                                av_ps[lo : lo + 64, 0:256],
                                lhsT=v[:, 0, hs],
                                rhs=pnorm[:, 0:256],
                                start=True,
                                stop=False,
                                skip_group_check=True,
                            )
                            nc.tensor.matmul(
                                av_ps[lo : lo + 64, 128:256],
                                lhsT=v[:, 1, hs],
                                rhs=pnorm[:, 256:384],
                                start=False,
                                stop=True,
                                skip_group_check=True,
                            )
                        nc.scalar.copy(out=attT[:, pr, :], in_=av_ps)
                    return th

                def mk_y(bi):
                    def th():
                        attT = attTs[bi]
                        for m in range(2):
                            y_ps = psa.tile([128, C], F32, tag="att", name=f"y_{pair}_{bi}_{m}")
                            for k in range(3):
                                nc.tensor.matmul(
                                    y_ps,
                                    lhsT=attT[:, k, bass.ts(m, 128)],
                                    rhs=wp_t[:, k, :],
                                    start=(k == 0),
                                    stop=(k == 2),
                                )
                            ysb = sb.tile([128, C], F32, tag="ysb", bufs=4, name=f"ysb_{pair}_{bi}_{m}")
                            nc.vector.tensor_add(out=ysb, in0=y_ps, in1=bb_t)
                            nc.sync.dma_start(
                                out=y[2 * pair + bi, bass.ts(m, 128), :], in_=ysb
                            )
                    return th

                for bi in range(2):
                    for pr in range(3):
                        thunks.append(mk_av(bi, pr))
                    thunks.append(mk_y(bi))
                return thunks

            if prev is not None:
                for th in prev:
                    th()
            prev = b_thunks(pair, vs, pnorms)

        if prev is not None:
            for th in prev:
                th()
    nc.compile()
    return nc


def pack_inputs(x, Wq, Wk, Wv, Wp, bp):
    """Host-side packing. Returns (common weight map, per-core xT shards)."""
    from einops import rearrange

    x = np.asarray(x, np.float32)
    Wq = np.asarray(Wq, np.float32)
    Wk = np.asarray(Wk, np.float32)
    Wv = np.asarray(Wv, np.float32)
    Wp = np.asarray(Wp, np.float32)
    bp = np.asarray(bp, np.float32)

    scale = 1.0 / np.sqrt(np.float32(HS))
    wq_h = rearrange(Wq * scale, "(p two) (k c) d -> c k p (two d)", two=2, k=3)
    wk_h = rearrange(Wk, "(p two) (k c) d -> c k p (two d)", two=2, k=3)
    wv_h = rearrange(Wv, "h (k c) d -> c k (h d)", k=3)
    wp_h = rearrange(Wp, "c2 (k c1) -> c1 k c2", k=3)

    # multiplicative causal mask for a diagonal [128,128] block of the
    # TRANSPOSED scores st[s, tq]: keep tq >= s, i.e. 1 if j >= i else 0
    msk_h = np.triu(np.ones((128, 128), np.float32))
    bb_h = np.tile(bp[None, :], (128, 1)).astype(np.float32)

    common = {
        "wq": np.ascontiguousarray(wq_h).astype(NPDT),
        "wk": np.ascontiguousarray(wk_h).astype(NPDT),
        "wv": np.ascontiguousarray(wv_h).astype(NPDT),
        "wp": np.ascontiguousarray(wp_h).astype(NPDT),
        "msk": msk_h.astype(NPDT),
        "bb": bb_h,
    }
    shards = []
    for c in range(NCORES):
        xs = x[c * BPC : (c + 1) * BPC]  # [BPC, T, C]
        # paired layout: [pair, kc, c_local, b'*T + t]
        xp = xs.reshape(BPC // 2, 2, T, C).transpose(0, 3, 1, 2)  # [pair, C, 2, T]
        xTs = xp.reshape(BPC // 2, 3, 128, 2 * T)
        shards.append(np.ascontiguousarray(xTs).astype(NPDT))
    return common, shards


_NC_CACHE = {}


def _get_nc(n_batch: int = BPC) -> bass.Bass:
    if n_batch not in _NC_CACHE:
        _NC_CACHE[n_batch] = build(n_batch)
    return _NC_CACHE[n_batch]


def kernel(x, Wq, Wk, Wv, Wp, bp):
    common, shards = pack_inputs(x, Wq, Wk, Wv, Wp, bp)
    nc = _get_nc()
    in_maps = [{**common, "xT": shards[c]} for c in range(NCORES)]
    res = run_bass_kernel_spmd(nc, in_maps, list(range(NCORES))).results
    y = np.concatenate([res[c]["y"] for c in range(NCORES)], axis=0)
    return np.ascontiguousarray(y.astype(np.float32))



# revision 8
# speedup vs baseline: 1.2002x; 1.2002x over previous
"""Multi-head causal self-attention (B=128, T=256, C=384, H=6, HS=64) for 8 TRN2 cores.

Strategy: pure data-parallel over batch (16 batch elements per core), weights
replicated, no collectives. Per batch element:

  - x^T (pre-transposed on host, [C, T]) is the shared rhs/lhsT for projections
  - Q^T, K^T computed per head-pair as [128(d), 256(t)] PSUM tiles (N=256 matmuls)
  - V computed in natural [t, (h d)] layout (rhs = all heads at once, N=384)
  - scores = Q^T.T-slices @ K^T with causal block-skipping:
      block(0,0) triangular [128,128], block(1,0) full, block(1,1) triangular;
      block(0,1) is never computed.
  - softmax without max-subtraction (scores bounded for this distribution):
      exp on ACT (one op per head over the packed [128, 384] score tile),
      multiplicative causal mask in ONE DVE op per head-pair (real const,
      no broadcast AP -- the GpSimd broadcast version was a 1.2us critical
      path producer that stalled the PE and caused HAM re-throttling).
  - row sums batched for all 12 heads of a pair into ONE [12, 256] PSUM tile
    (unit-column lhsT), so reciprocal+cast are 2 DVE ops per pair, not 18.
  - normalization deferred to the AV->attT copy: a [12->128] selector matmul
    broadcasts r per head ([128, 256], N=256, contraction 12), then the
    PSUM->SBUF attT copy is a fused DVE multiply. This kills the old
    [128,512] ones-broadcast matmuls and the separate pnorm multiplies.
  - AV runs on the UNNORMALIZED masked exp scores, accumulated as [d, t]
    directly into the concat-head layout att^T (normalized at the copy).
  - y = att^T.T @ Wp^T + bp, bias fused into the PSUM->SBUF copy on DVE.

Emission order per pair p: QKV(p), scores+exp+mask(p), stage-B(p-1)
[AV/bcast/attT/Y], sums(p), recip(p). The PE never waits on a slow
cross-engine producer, so HAM stays at 8/8 (the baseline oscillated
8/8 <-> 4/8 every pair, spending 49% of the run at half clock).

Matmul operands in bf16 (fp32 PSUM accumulation), softmax stats in fp32.
"""

import numpy as np
import ml_dtypes
from contextlib import ExitStack

import concourse.bass as bass
import concourse.bacc as bacc
import concourse.mybir as mybir
import concourse.tile as tile
from concourse.bass_utils import run_bass_kernel_spmd

B, T, C, H, HS = 128, 256, 384, 6, 64
NCORES = 8
BPC = B // NCORES  # batch elements per core

F32 = mybir.dt.float32
DT = mybir.dt.bfloat16
NPDT = ml_dtypes.bfloat16

EXP = mybir.ActivationFunctionType.Exp
MUL = mybir.AluOpType.mult
ADD = mybir.AluOpType.add


def build(n_batch: int = BPC) -> bass.Bass:
    assert n_batch % 2 == 0
    npair = n_batch // 2
    nc = bacc.Bacc("TRN2", target_bir_lowering=False, debug=False)

    xT = nc.dram_tensor("xT", [npair, 3, 128, 2 * T], DT, kind="ExternalInput").ap()
    wq = nc.dram_tensor("wq", [128, 3, 3, 128], DT, kind="ExternalInput").ap()
    wk = nc.dram_tensor("wk", [128, 3, 3, 128], DT, kind="ExternalInput").ap()
    wv = nc.dram_tensor("wv", [128, 3, C], DT, kind="ExternalInput").ap()
    wp = nc.dram_tensor("wp", [128, 3, C], DT, kind="ExternalInput").ap()
    mskF = nc.dram_tensor("mskF", [128, 2, 128], DT, kind="ExternalInput").ap()
    ej = nc.dram_tensor("ej", [128, 12, 12], DT, kind="ExternalInput").ap()
    sel = nc.dram_tensor("sel", [12, 6, 128], DT, kind="ExternalInput").ap()
    bb = nc.dram_tensor("bb", [128, C], F32, kind="ExternalInput").ap()
    y = nc.dram_tensor("y", [n_batch, T, C], F32, kind="ExternalOutput").ap()

    with tile.TileContext(nc) as tc, ExitStack() as ctx:
        const = ctx.enter_context(tc.tile_pool(name="const", bufs=1))
        sb = ctx.enter_context(tc.tile_pool(name="sb", bufs=2))
        # uniform pool: every PSUM tile here is <= 1 bank
        psa = ctx.enter_context(tc.tile_pool(name="psa", bufs=8, space="PSUM"))

        wq_t = const.tile([128, 3, 3, 128], DT)
        nc.sync.dma_start(out=wq_t, in_=wq)
        wk_t = const.tile([128, 3, 3, 128], DT)
        nc.sync.dma_start(out=wk_t, in_=wk)
        wv_t = const.tile([128, 3, C], DT)
        nc.gpsimd.dma_start(out=wv_t, in_=wv)
        wp_t = const.tile([128, 3, C], DT)
        nc.gpsimd.dma_start(out=wp_t, in_=wp)
        mskF_t = const.tile([128, 2, 128], DT)
        nc.gpsimd.dma_start(out=mskF_t, in_=mskF)
        ej_t = const.tile([128, 12, 12], DT)
        nc.gpsimd.dma_start(out=ej_t, in_=ej)
        sel_t = const.tile([12, 6, 128], DT)
        nc.gpsimd.dma_start(out=sel_t, in_=sel)
        bb_t = const.tile([128, C], F32)
        nc.gpsimd.dma_start(out=bb_t, in_=bb)

        # HAM warm-up: ~7us of back-to-back dummy matmuls during the initial
        # weight/x DMA wait, so the PE clock is at 8/8 when real work starts.
        # Same-engine WAW chain -> no cross-engine waits, PE runs them densely.
        warm_in = const.tile([128, 512], DT)
        nc.vector.memset(warm_in, 0.0)
        warm_ps = psa.tile([128, 512], F32, tag="att")
        for _ in range(30):
            nc.tensor.matmul(
                warm_ps, lhsT=warm_in[:, 0:128], rhs=warm_in,
                start=True, stop=True,
            )

        def stage_a(pair):
            """QKV projections, scores, exp, mask for one pair."""
            xt = sb.tile([128, 3, 2 * T], DT, tag="xt", bufs=4)
            nc.sync.dma_start(out=xt, in_=xT[pair].rearrange("k c t -> c k t"))

            # Q^T / K^T for both batch elems of the pair (N=512), per head
            # pair; V per batch elem in natural [t, (h d)] layout
            qt = sb.tile([128, 3, 2 * T], DT, tag="qt")
            kt = sb.tile([128, 3, 2 * T], DT, tag="kt")
            for p in range(3):
                qt_ps = psa.tile([128, 2 * T], F32, tag="att")
                for k in range(3):
                    nc.tensor.matmul(
                        qt_ps,
                        lhsT=wq_t[:, k, p, :],
                        rhs=xt[:, k, :],
                        start=(k == 0),
                        stop=(k == 2),
                    )
                nc.scalar.copy(out=qt[:, p, :], in_=qt_ps)
                kt_ps = psa.tile([128, 2 * T], F32, tag="att")
                for k in range(3):
                    nc.tensor.matmul(
                        kt_ps,
                        lhsT=wk_t[:, k, p, :],
                        rhs=xt[:, k, :],
                        start=(k == 0),
                        stop=(k == 2),
                    )
                nc.scalar.copy(out=kt[:, p, :], in_=kt_ps)
            vs = []
            for bi in range(2):
                v = sb.tile([128, 2, C], DT, tag="v", bufs=6)
                for m in range(2):
                    v_ps = psa.tile([128, C], F32, tag="att")
                    for k in range(3):
                        nc.tensor.matmul(
                            v_ps,
                            lhsT=xt[:, k, bi * T + m * 128 : bi * T + (m + 1) * 128],
                            rhs=wv_t[:, k, :],
                            start=(k == 0),
                            stop=(k == 2),
                        )
                    nc.scalar.copy(out=v[:, m, :], in_=v_ps)
                vs.append(v)

            # transposed scores st[s, tq] + exp + causal mask;
            # the two heads of a pair share one packed pex SBUF tile
            pexs = {}
            for bi in range(2):
                for pr in range(3):
                    pex = sb.tile([128, 2, 384], DT, tag="pex", bufs=16)
                    for two in range(2):
                        h = 2 * pr + two
                        lo = two * 64
                        qh = qt[lo : lo + 64, pr, bi * T : (bi + 1) * T]
                        kh = kt[lo : lo + 64, pr, bi * T : (bi + 1) * T]
                        # packed [s0 x tq(0:256) | s1 x tq(128:256)]
                        st = psa.tile([128, 384], F32, tag="att")
                        nc.tensor.matmul(
                            st[:, 0:256],
                            lhsT=kh[:, 0:128],
                            rhs=qh,
                            start=True,
                            stop=True,
                        )
                        nc.tensor.matmul(
                            st[:, 256:384],
                            lhsT=kh[:, 128:256],
                            rhs=qh[:, 128:256],
                            start=True,
                            stop=True,
                        )
                        # exp (scores bounded, no max trick)
                        nc.scalar.activation(out=pex[:, two, :], in_=st, func=EXP)
                    # multiplicative causal mask (keep tq >= s) on the four
                    # triangular blocks of the packed 2-head pex: two 3D DVE
                    # ops (DVE tensor_tensor is S3S3D3 -- partition + 2 free
                    # dims max; a 4D AP crashes the exec unit) against a REAL
                    # [128, 2, 128] mask constant (same triangle both heads).
                    nc.vector.tensor_mul(
                        out=pex[:, :, 0:128], in0=pex[:, :, 0:128], in1=mskF_t
                    )
                    nc.vector.tensor_mul(
                        out=pex[:, :, 256:384], in0=pex[:, :, 256:384], in1=mskF_t
                    )
                    pexs[(bi, 2 * pr)] = pex[:, 0, :]
                    pexs[(bi, 2 * pr + 1)] = pex[:, 1, :]
                    pexs[(bi, "t", pr)] = pex
            return vs, pexs

        def stage_sums(pair, pexs):
            """Row sums for all 12 heads of the pair into one [12, 256] PSUM
            tile (unit-column lhsT), then one reciprocal + one bf16 cast."""
            sums12 = psa.tile([12, 256], F32, tag="att", name=f"sums_{pair}")
            first = True
            for bi in range(2):
                for pr in range(3):
                    pex2 = pexs[(bi, "t", pr)]  # [128, 2, 384]
                    for two in range(2):
                        h = 6 * bi + 2 * pr + two
                        nc.tensor.matmul(
                            sums12,
                            lhsT=ej_t[:, h, :],
                            rhs=pex2[:, two, 0:256],
                            start=first,
                            stop=False,
                            skip_group_check=True,
                        )
                        first = False
            for bi in range(2):
                for pr in range(3):
                    pex2 = pexs[(bi, "t", pr)]
                    for two in range(2):
                        h = 6 * bi + 2 * pr + two
                        last = bi == 1 and pr == 2 and two == 1
                        nc.tensor.matmul(
                            sums12[:, 128:256],
                            lhsT=ej_t[:, h, :],
                            rhs=pex2[:, two, 256:384],
                            start=False,
                            stop=last,
                            skip_group_check=True,
                        )
            rscr = sb.tile([12, 256], F32, tag="rscr", bufs=3)
            nc.vector.reciprocal_approx_fast(out=rscr, in_=sums12)
            rsb = sb.tile([12, 256], DT, tag="rsb", bufs=3)
            nc.vector.tensor_copy(out=rsb, in_=rscr)
            return rsb

        def stage_b(pair, vs, pexs, rsb):
            """AV (unnormalized), per-head r broadcast via selector matmul,
            fused normalize on the attT copy, output projection."""
            attTs = {}
            for bi in range(2):
                v = vs[bi]
                attTs[bi] = sb.tile(
                    [128, 3, 256], DT, tag="attT", bufs=4, name=f"attT_{pair}_{bi}"
                )
                attT = attTs[bi]
                for pr in range(3):
                    av_ps = psa.tile([128, 256], F32, tag="att", name=f"av_{pair}_{bi}_{pr}")
                    for two in range(2):
                        h = 2 * pr + two
                        lo = two * 64
                        pexh = pexs[(bi, h)]
                        hs = slice(h * 64, h * 64 + 64)
                        nc.tensor.matmul(
                            av_ps[lo : lo + 64, 0:256],
                            lhsT=v[:, 0, hs],
                            rhs=pexh[:, 0:256],
                            start=True,
                            stop=False,
                            skip_group_check=True,
                        )
                        nc.tensor.matmul(
                            av_ps[lo : lo + 64, 128:256],
                            lhsT=v[:, 1, hs],
                            rhs=pexh[:, 256:384],
                            start=False,
                            stop=True,
                            skip_group_check=True,
                        )
                    # r broadcast: bc[i, tq] = r[2j + (i>=64), tq], j = head
                    # pair index within the batch pair
                    j = 3 * bi + pr
                    bc_ps = psa.tile([128, 256], F32, tag="att", name=f"bc_{pair}_{bi}_{pr}")
                    nc.tensor.matmul(
                        bc_ps,
                        lhsT=sel_t[:, j, :],
                        rhs=rsb,
                        start=True,
                        stop=True,
                    )
                    bc_sb = sb.tile([128, 256], DT, tag="bcsb", bufs=6, name=f"bcs_{pair}_{bi}_{pr}")
                    nc.scalar.copy(out=bc_sb, in_=bc_ps)
                    nc.vector.tensor_mul(out=attT[:, pr, :], in0=av_ps, in1=bc_sb)
            for bi in range(2):
                attT = attTs[bi]
                for m in range(2):
                    y_ps = psa.tile([128, C], F32, tag="att", name=f"y_{pair}_{bi}_{m}")
                    for k in range(3):
                        nc.tensor.matmul(
                            y_ps,
                            lhsT=attT[:, k, bass.ts(m, 128)],
                            rhs=wp_t[:, k, :],
                            start=(k == 0),
                            stop=(k == 2),
                        )
                    ysb = sb.tile([128, C], F32, tag="ysb", bufs=4, name=f"ysb_{pair}_{bi}_{m}")
                    nc.vector.tensor_add(out=ysb, in0=y_ps, in1=bb_t)
                    nc.sync.dma_start(
                        out=y[2 * pair + bi, bass.ts(m, 128), :], in_=ysb
                    )

        prev = None
        for pair in range(npair):
            vs, pexs = stage_a(pair)
            if prev is not None:
                stage_b(*prev)
            rsb = stage_sums(pair, pexs)
            prev = (pair, vs, pexs, rsb)
        if prev is not None:
            stage_b(*prev)
    nc.compile()
    return nc


def pack_inputs(x, Wq, Wk, Wv, Wp, bp):
    """Host-side packing. Returns (common weight map, per-core xT shards)."""
    from einops import rearrange

    x = np.asarray(x, np.float32)
    Wq = np.asarray(Wq, np.float32)
    Wk = np.asarray(Wk, np.float32)
    Wv = np.asarray(Wv, np.float32)
    Wp = np.asarray(Wp, np.float32)
    bp = np.asarray(bp, np.float32)

    scale = 1.0 / np.sqrt(np.float32(HS))
    wq_h = rearrange(Wq * scale, "(p two) (k c) d -> c k p (two d)", two=2, k=3)
    wk_h = rearrange(Wk, "(p two) (k c) d -> c k p (two d)", two=2, k=3)
    wv_h = rearrange(Wv, "h (k c) d -> c k (h d)", k=3)
    wp_h = rearrange(Wp, "c2 (k c1) -> c1 k c2", k=3)

    # multiplicative causal mask for a diagonal [128,128] block of the
    # TRANSPOSED scores st[s, tq]: keep tq >= s, i.e. 1 if j >= i else 0;
    # materialized [128, 2(head), 2(block), 128] so the DVE op needs no
    # broadcast access pattern
    tri = np.triu(np.ones((128, 128), np.float32))
    mskF_h = np.broadcast_to(tri[:, None, :], (128, 2, 128)).copy()
    # unit-column matrices for the batched row-sum matmuls:
    # ej[:, h, i] = 1 iff i == h  (lhsT [128, 12] with ones in column h)
    ej_h = np.broadcast_to(np.eye(12, dtype=np.float32)[None, :, :], (128, 12, 12)).copy()
    # selector for the per-head r broadcast: sel[p, j, i] = 1 iff
    # p == 2j + (i >= 64)
    sel_h = np.zeros((12, 6, 128), np.float32)
    for j in range(6):
        sel_h[2 * j, j, 0:64] = 1.0
        sel_h[2 * j + 1, j, 64:128] = 1.0
    bb_h = np.tile(bp[None, :], (128, 1)).astype(np.float32)

    common = {
        "wq": np.ascontiguousarray(wq_h).astype(NPDT),
        "wk": np.ascontiguousarray(wk_h).astype(NPDT),
        "wv": np.ascontiguousarray(wv_h).astype(NPDT),
        "wp": np.ascontiguousarray(wp_h).astype(NPDT),
        "mskF": mskF_h.astype(NPDT),
        "ej": ej_h.astype(NPDT),
        "sel": sel_h.astype(NPDT),
        "bb": bb_h,
    }
    shards = []
    for c in range(NCORES):
        xs = x[c * BPC : (c + 1) * BPC]  # [BPC, T, C]
        # paired layout: [pair, kc, c_local, b'*T + t]
        xp = xs.reshape(BPC // 2, 2, T, C).transpose(0, 3, 1, 2)  # [pair, C, 2, T]
        xTs = xp.reshape(BPC // 2, 3, 128, 2 * T)
        shards.append(np.ascontiguousarray(xTs).astype(NPDT))
    return common, shards


_NC_CACHE = {}


def _get_nc(n_batch: int = BPC) -> bass.Bass:
    if n_batch not in _NC_CACHE:
        _NC_CACHE[n_batch] = build(n_batch)
    return _NC_CACHE[n_batch]


def kernel(x, Wq, Wk, Wv, Wp, bp):
    common, shards = pack_inputs(x, Wq, Wk, Wv, Wp, bp)
    nc = _get_nc()
    in_maps = [{**common, "xT": shards[c]} for c in range(NCORES)]
    res = run_bass_kernel_spmd(nc, in_maps, list(range(NCORES))).results
    y = np.concatenate([res[c]["y"] for c in range(NCORES)], axis=0)
    return np.ascontiguousarray(y.astype(np.float32))
